# revision 1
# baseline (speedup 1.0000x reference)
"""Trainium kernel for nn_Cif_52922587021904 (CIF segment_reduce).

Strategy (pure data parallel per sharding hint): shard B=16 across the 8
NeuronCores (2 sequences each), run the whole per-sample pipeline on-device
via one jitted pmap program, gather full outputs on host.

Key optimization vs the reference formulation: the conv1d output y is only
ever consumed through the linear projection lin_w (alpha = sigmoid(lin_w.y)),
so we fold lin_w into the conv weights on the host:
    v[cin, k] = sum_cout lin_w[cout] * conv_w[cout, cin, k]
    z[b, t]   = sum_{cin,k} v[cin,k] * hs[b, t+k-2, cin] + c0
This reduces the matmul work from B*T*C*C*K (4.3e10 MAC) to B*T*C*K
(1.7e8 MAC) with ~1e-6 absolute error in z (validated: output rel err
contribution << 1e-3). The remaining work - cumsum, per-frame weight split,
segment scatter-add, masking - is memory-bound, matching target_regime.
"""

import numpy as np
import jax
import jax.numpy as jnp
from functools import partial

TH = 1.0
B, T, C, K = 16, 8192, 256, 5
S = T + 1
N_CORES = 8
SEQ_PER_CORE = B // N_CORES


def _cif_single(alpha, h):
    """Vectorized integrate-and-fire for one sequence (matches reference)."""
    csum = jnp.cumsum(alpha)
    csum_prev = csum - alpha
    k0 = jnp.floor(csum_prev / TH).astype(jnp.int32)
    k1 = jnp.floor(csum / TH).astype(jnp.int32)
    w0 = jnp.minimum(csum, (k0 + 1).astype(alpha.dtype) * TH) - csum_prev
    w1 = jnp.where(k1 > k0, csum - k1.astype(alpha.dtype) * TH, jnp.zeros_like(alpha))
    # two scatter-adds instead of reference's concat (half the scatter traffic;
    # per-segment f32 add-order differences are ~ulp level)
    out = jax.ops.segment_sum(w0[:, None] * h, k0, num_segments=S)
    out = out + jax.ops.segment_sum(w1[:, None] * h, k1, num_segments=S)
    total = csum[-1]
    n_fire = jnp.floor(total / TH).astype(jnp.int32)
    residual = total - n_fire.astype(alpha.dtype) * TH
    keep_tail = residual >= TH / 2
    idx = jnp.arange(S)
    mask = (idx < n_fire) | ((idx == n_fire) & keep_tail)
    return out * mask[:, None].astype(out.dtype), total


@partial(jax.pmap, static_broadcasted_argnums=())
def _per_core(hs, v, c0):
    # hs: (SEQ_PER_CORE, T, C); v: (C, K); c0: scalar (1,)-shaped
    # z[b, t] = sum_k sum_cin v[cin, k] * hs[b, t + k - 2, cin] + c0
    hs_p = jnp.pad(hs, ((0, 0), (2, 2), (0, 0)))
    z = c0[0]
    for k in range(K):
        z = z + jnp.einsum("btc,c->bt", jax.lax.dynamic_slice_in_dim(hs_p, k, T, axis=1), v[:, k])
    alpha = jax.nn.sigmoid(z)
    cs, totals = jax.vmap(_cif_single)(alpha, hs)
    return cs, totals


def kernel(hs_pad, hs_mask, conv_w, conv_b, lin_w, lin_b):
    # host-side weight folding in f64 (tiny: 256*256*5)
    v = np.einsum("o,oik->ik", lin_w.astype(np.float64), conv_w.astype(np.float64))
    v = np.ascontiguousarray(v, dtype=np.float32)  # (C, K)
    c0 = np.float32(np.dot(lin_w.astype(np.float64), conv_b.astype(np.float64))
                    + np.float64(lin_b))

    devices = jax.devices()[:N_CORES]
    hs_sharded = np.ascontiguousarray(
        hs_pad.reshape(N_CORES, SEQ_PER_CORE, T, C), dtype=np.float32)
    v_rep = np.broadcast_to(v, (N_CORES,) + v.shape)
    c0_rep = np.broadcast_to(np.asarray([c0], np.float32), (N_CORES, 1))

    cs, totals = _per_core(
        jax.device_put_sharded(list(hs_sharded), devices),
        jax.device_put_sharded(list(v_rep), devices),
        jax.device_put_sharded(list(c0_rep), devices),
    )
    cs = np.asarray(cs, dtype=np.float32).reshape(B, S, C)
    totals = np.asarray(totals, dtype=np.float64).reshape(B)
    loss_pen = np.float32(np.abs(totals).sum())
    return cs, loss_pen
